# revision 72
# baseline (speedup 1.0000x reference)
"""Trainium2 Bass kernel: fused multi-head attention block (projections +
softmax attention + output projection + residual + LayerNorm).

Sharding: 8 cores = 2 batches x 4 query-chunks of 512. Each core computes
K/V for its whole batch (replicated within the 4-core batch group), Q only
for its 512-query chunk, full attention for that chunk over all 16 heads,
the output projection, residual add and LayerNorm. No collectives.

All cores run the same program; per-core inputs are pre-sliced on the host
with the key/value token order ROTATED so the core's query chunk occupies
rows 0..511 (attention is permutation-invariant over keys, and the key
padding mask is rotated identically).

The d'-tile loop is software-pipelined at key-tile granularity: each jj
step emits logits(dt), two K-projection chain steps for dt+1, and PV
chain steps for dt (consuming exp output just-in-time), so the PE stays
gapless while the Scalar engine's exp stream drains the logits PSUM.

Device-side layouts (per core):
  xt   [1024, 2048] bf16  x[b] transposed (feature-major), rotated
  xq   [512, 1024]  f32   query-chunk rows of x[b] (residual input)
  wq/wk/wv [1024, 1024] bf16  [c, h*64] (head-minor)
  wo   [1024, 1024] bf16  [(h*64+d), m]
  bias [16, 128]    f32   additive key mask bias per key tile/partition
  gamma/beta [1024] bf16
Output: y [512, 1024] f32.
"""

import contextlib

import numpy as np
import ml_dtypes

import concourse.bass as bass
import concourse.tile as tile
from concourse import mybir
from concourse import bass_utils

BF16 = ml_dtypes.bfloat16
N_CORES = 8
B, L, D, H, DH = 2, 2048, 1024, 16, 64
Q = L // 4          # queries per core
CT = D // 128       # contraction tiles over features
JT = L // 128       # key tiles
IT = Q // 128       # query tiles
LN_EPS = 1e-5

F32 = mybir.dt.float32
BF = mybir.dt.bfloat16


def _split_waits(nc, maxw=1):
    """This walrus build rejects instructions with more than one sync wait;
    split excess waits into preceding NOPs on the same engine."""
    ctr = 0
    for fn in nc.m.functions:
        for bb in fn.blocks:
            new_insts = []
            for inst in bb.instructions:
                si = inst.sync_info
                if si is not None and len(si.on_wait) > maxw:
                    waits = list(si.on_wait)
                    excess, keep = waits[:-maxw], waits[-maxw:]
                    for i in range(0, len(excess), maxw):
                        ctr += 1
                        new_insts.append(mybir.InstNoOp(
                            name=f"waitsplit_nop_{ctr}",
                            engine=inst.engine,
                            sync_info=mybir.SyncInfo(
                                on_wait=excess[i:i + maxw], on_update=[]),
                            text_hint="waitsplit",
                        ))
                    si.on_wait = keep
                new_insts.append(inst)
            bb.instructions = new_insts
    return ctr


def _emit(nc, tc, hh, masked, ln_plain=False):
    Exp = mybir.ActivationFunctionType.Exp
    Sqrt = mybir.ActivationFunctionType.Sqrt

    xt_ap = hh["xt"].ap().rearrange("(t p) l -> p t l", p=128)      # [128,8,2048]
    wq_ap = hh["wq"].ap().rearrange("(t p) d -> p t d", p=128)
    wk_ap = hh["wk"].ap().rearrange("(t p) d -> p t d", p=128)
    wv_ap = hh["wv"].ap().rearrange("(t p) d -> p t d", p=128)
    wo_ap = hh["wo"].ap().rearrange("(t p) d -> p t d", p=128)
    bias_ap = hh["bias"].ap().rearrange("a b -> b a")               # [128,16]
    xq_ap = hh["xq"].ap()
    y_ap = hh["y"].ap()

    def bcast_dram(h1d, parts=128):
        a = h1d.ap()
        return bass.AP(tensor=a.tensor, offset=a.offset,
                       ap=[[0, parts]] + list(a.ap))

    with contextlib.ExitStack() as ctx:
        const = ctx.enter_context(tc.tile_pool(name="const", bufs=1))
        wpool = ctx.enter_context(tc.tile_pool(name="wpool", bufs=2))
        xtp = ctx.enter_context(tc.tile_pool(name="xtp", bufs=2))
        expp = ctx.enter_context(tc.tile_pool(name="expp", bufs=2))
        ktp = ctx.enter_context(tc.tile_pool(name="ktp", bufs=2))
        vp = ctx.enter_context(tc.tile_pool(name="vp", bufs=1))
        qtp = ctx.enter_context(tc.tile_pool(name="qtp", bufs=1))
        ptp = ctx.enter_context(tc.tile_pool(name="ptp", bufs=1))
        npool = ctx.enter_context(tc.tile_pool(name="npool", bufs=3))
        xqp = ctx.enter_context(tc.tile_pool(name="xqp", bufs=4))
        lnp = ctx.enter_context(tc.tile_pool(name="lnp", bufs=3))
        statp = ctx.enter_context(tc.tile_pool(name="statp", bufs=4))

        # ---- constants / small loads ----
        eps_sb = const.tile([128, 1], F32)
        nc.vector.memset(eps_sb[:], LN_EPS)
        ones64 = const.tile([33, 64], BF)
        nc.vector.memset(ones64[:], 1.0)
        den2_bufs = []
        for i in range(2):
            d2 = const.tile([33, 512], F32, name=f"den2buf{i}")
            nc.vector.memset(d2[:], 1.0)
            den2_bufs.append(d2)
        bias_sb = const.tile([128, 16], F32)
        nc.gpsimd.dma_start(out=bias_sb[:], in_=bias_ap)
        gamma_sb = const.tile([128, 1024], BF)
        beta_sb = const.tile([128, 1024], BF)

        # ---- big SBUF tensors ----
        v_all = vp.tile([128, JT, H, DH + 1], BF)  # V by key tile, +ones col
        qt_all = qtp.tile([128, 8, Q], BF)         # Q^T for the chunk
        probt = ptp.tile([128, 8, Q], BF)          # normalized P^T stacked

        nc.vector.memset(v_all[:, :, :, DH:DH + 1], 1.0)

        # x is the gate for the first projections: split it across the SP
        # and ACT DMA queues (1MB halves land first), and push the weights
        # to the DVE/POOL queues so they stream in parallel.
        # startup is DMA-bandwidth-bound: wv halves land first on the two
        # HWDGE queues, then x arrives in key-column chunks matched to the
        # V-chain consumption order (chain lt needs x columns lt*128..),
        # so the PE starts ~17us instead of waiting for all 6MB.
        xt0 = xtp.tile([128, 4, 2048], BF, tag="xt")
        xt1 = xtp.tile([128, 4, 2048], BF, tag="xt")
        wv_sb = wpool.tile([128, 8, 1024], BF, tag="w")
        nc.sync.dma_start(out=wv_sb[:, :, 0:512], in_=wv_ap[:, :, 0:512])
        nc.scalar.dma_start(out=wv_sb[:, :, 512:1024],
                            in_=wv_ap[:, :, 512:1024])
        for c in range(4):
            lo, hi = c * 512, (c + 1) * 512
            nc.sync.dma_start(out=xt0[:, :, lo:hi],
                              in_=xt_ap[:, 0:4, lo:hi])
            nc.scalar.dma_start(out=xt1[:, :, lo:hi],
                                in_=xt_ap[:, 4:8, lo:hi])
        wq_sb = wpool.tile([128, 8, 1024], BF, tag="w")
        nc.gpsimd.dma_start(out=wq_sb[:], in_=wq_ap)

        def xt_sl(ct, lo, size):
            t = xt0 if ct < 4 else xt1
            return t[:, ct % 4, lo:lo + size]

        kt_tiles = {}

        with tc.tile_pool(name="psA", bufs=3, space="PSUM") as psA:
            # ---- V projection: [token 128][h*64] ----
            for lt in range(JT):
                ps = psA.tile([128, 2, 512], F32, tag="aa")
                for nt in range(2):
                    for ct in range(CT):
                        nc.tensor.matmul(
                            ps[:, nt, :], xt_sl(ct, lt * 128, 128),
                            wv_sb[:, ct, nt * 512:(nt + 1) * 512],
                            start=(ct == 0), stop=(ct == CT - 1))
                nc.vector.tensor_copy(
                    v_all[:, lt, :, 0:DH],
                    ps.rearrange("p n (h d) -> p (n h) d", h=8))

            wk_sb = wpool.tile([128, 8, 1024], BF, tag="w")
            nc.gpsimd.dma_start(out=wk_sb[:], in_=wk_ap)

            # ---- K^T projection for dt=0, first half (so its eviction
            # lands during Q-proj and logits(0) never waits) ----
            kt0 = ktp.tile([128, 2048], BF, tag="kt")
            kt_tiles[0] = kt0

            def k0_half(jp):
                ps = psA.tile([128, 2, 512], F32, tag="aa",
                              name=f"k0p{jp}")
                for j4 in range(2):
                    for ct in range(CT):
                        nc.tensor.matmul(
                            ps[:, j4, :],
                            wk_sb[:, ct, 0:128],
                            xt_sl(ct, jp * 1024 + j4 * 512, 512),
                            start=(ct == 0), stop=(ct == CT - 1))
                nc.vector.tensor_copy(
                    kt0[:, jp * 1024:(jp + 1) * 1024], ps[:])

            # ---- Q^T projection: [d' 128][i 512] ----
            for dtp in range(4):
                ps = psA.tile([128, 2, 512], F32, tag="aa")
                for half in range(2):
                    dt = 2 * dtp + half
                    for ct in range(CT):
                        nc.tensor.matmul(
                            ps[:, half, :],
                            wq_sb[:, ct, dt * 128:(dt + 1) * 128],
                            xt_sl(ct, 0, Q),
                            start=(ct == 0), stop=(ct == CT - 1))
                nc.vector.tensor_copy(qt_all[:, 2 * dtp:2 * dtp + 2, :], ps[:])

            wo_sb = wpool.tile([128, 8, 1024], BF, tag="w")
            nc.gpsimd.dma_start(out=wo_sb[:], in_=wo_ap)

            k0_half(0)
            k0_half(1)

        # ---- attention: software-pipelined over (dt, jj) ----
        # per jj step: logits(dt, jj) x2hb, two K-proj(dt+1) chain steps,
        # PV(dt, jj-2) x2hb; exp on Scalar drains logits PSUM. The
        # denominator path is decoupled from every PSUM pool the PE loops
        # on: PV PSUM is staged to SBUF right away (freeing psP), the slow
        # fixed-cost Vector reciprocal runs SBUF-only, and the PE broadcast
        # + normalize mul for dt are emitted inside dt+1's jj stream, by
        # which point the reciprocal is long done. K evictions go on
        # Scalar so Vector's reciprocals never delay them.
        with tc.tile_pool(name="psL", bufs=2, space="PSUM") as psL, \
             tc.tile_pool(name="psK", bufs=1, space="PSUM") as psK, \
             tc.tile_pool(name="psP", bufs=2, space="PSUM") as psP, \
             tc.tile_pool(name="psD", bufs=1, space="PSUM") as psD:
            den_prev = None
            for dt in range(8):
                kt_cur = kt_tiles.pop(dt)
                if dt < 7:
                    kt_nxt = ktp.tile([128, 2048], BF, tag="kt")
                    kt_tiles[dt + 1] = kt_nxt
                expt_h = [expp.tile([128, JT // 2, 2, 512], BF, tag="e",
                                    name=f"expt{dt}_{h}") for h in range(2)]

                def expt(jj):
                    return expt_h[jj // (JT // 2)][:, jj % (JT // 2), :, :]
                pv_ps = [psP.tile([DH + 1, 512], F32, tag="pp",
                                  name=f"pv{dt}_{hb}") for hb in range(2)]
                kstate = {}

                def kstep(s):
                    # chain c = s//8 covers keys [c*512, (c+1)*512), ct = s%8
                    c, ct = s // 8, s % 8
                    if ct == 0:
                        kstate[c] = psK.tile([128, 512], F32, tag="kk",
                                             name=f"k{dt}_{c}")
                    nc.tensor.matmul(
                        kstate[c][:],
                        wk_sb[:, ct, (dt + 1) * 128:(dt + 2) * 128],
                        xt_sl(ct, c * 512, 512),
                        start=(ct == 0), stop=(ct == CT - 1))
                    if ct == CT - 1:
                        # chains whose eviction gates the next psK alloc
                        # tightly go on Scalar; the slack ones on Vector
                        # (whose queue may be behind a 3.3us reciprocal).
                        if c % 2 == 0:
                            nc.scalar.copy(
                                kt_nxt[:, c * 512:(c + 1) * 512],
                                kstate.pop(c))
                        else:
                            nc.vector.tensor_copy(
                                kt_nxt[:, c * 512:(c + 1) * 512],
                                kstate.pop(c))

                def pvstep(jj):
                    for hb in range(2):
                        nc.tensor.matmul(
                            pv_ps[hb][:], v_all[:, jj, 2 * dt + hb, 0:DH + 1],
                            expt(jj)[:, hb, :],
                            start=(jj == 0), stop=(jj == JT - 1))

                def den_drain(step):
                    # finish dt-1's normalization: PE broadcasts 1/den, then
                    # Vector scales the staged P^T into probt.
                    if den_prev is None:
                        return
                    d, pvsb, rdiv, ps_d = den_prev
                    if step == 0:
                        nc.tensor.matmul(ps_d[0:64, :], ones64[0:1, :],
                                         rdiv[0], start=True, stop=True)
                    elif step == 1:
                        nc.tensor.matmul(ps_d[64:128, :], ones64[32:33, :],
                                         rdiv[1], start=True, stop=True)
                    elif step == 2:
                        nc.vector.tensor_mul(probt[0:64, d, :],
                                             pvsb[0][0:DH, :], ps_d[0:64, :])
                    else:
                        nc.vector.tensor_mul(probt[64:128, d, :],
                                             pvsb[1][0:DH, :],
                                             ps_d[64:128, :])

                for jj in range(JT):
                    if jj >= 2:
                        pvstep(jj - 2)
                    psq = psL.tile([128, 2, 512], F32, tag="ll")
                    # interleave logits and K chain steps so every
                    # 128-column LDWEIGHTS shadows a full-length matmul
                    for hb in range(2):
                        nc.tensor.matmul(
                            psq[:, hb, :],
                            kt_cur[hb * 64:hb * 64 + 64,
                                   jj * 128:(jj + 1) * 128],
                            qt_all[hb * 64:hb * 64 + 64, dt, :],
                            start=True, stop=True)
                        if dt < 7:
                            kstep(2 * jj + hb)
                    if masked:
                        for hb in range(2):
                            nc.scalar.activation(
                                expt(jj)[:, hb, :], psq[:, hb, :], Exp,
                                bias=bias_sb[:, jj:jj + 1], scale=1.0 / 8.0)
                    else:
                        nc.scalar.activation(
                            expt(jj).rearrange("p a b -> p (a b)"),
                            psq.rearrange("p a b -> p (a b)"), Exp,
                            bias=0.0, scale=1.0 / 8.0)
                    if jj in (6, 8, 10, 12):
                        den_drain((jj - 6) // 2)
                pvstep(JT - 2)
                pvstep(JT - 1)

                # stage PV PSUM to SBUF (frees psP) and take the reciprocal
                # off-path; the broadcast+mul happen during dt+1. The DVE
                # reciprocal has a ~3.3us fixed cost, so both heads'
                # denominators share one [1,1024] instruction.
                pvsb = []
                den2 = den2_bufs[dt % 2]
                for hb in range(2):
                    t = npool.tile([DH + 1, 512], F32, tag="pv",
                                   name=f"pvsb{dt}_{hb}")
                    nc.vector.tensor_copy(t[:], pv_ps[hb][:])
                    pvsb.append(t)
                    nc.vector.tensor_copy(den2[32 * hb:32 * hb + 1, :],
                                          t[DH:DH + 1, :])
                # reciprocal cost scales with free size only, so one [33,512]
                # instruction covers both heads (rows 1..31 are don't-care).
                rdivw = npool.tile([33, 512], BF, tag="n",
                                   name=f"rdivw{dt}")
                with nc.allow_low_precision(
                        reason="bf16 1/den: per-(h,q) scale, ~0.2%"):
                    nc.vector.reciprocal(rdivw[:], den2[:])
                rdiv = [rdivw[0:1, :], rdivw[32:33, :]]
                if dt < 7:
                    ps_d = psD.tile([128, 512], F32, tag="dd")
                else:
                    ps_d = None  # dt=7 drains in the tail from the psO pool
                den_prev = (dt, pvsb, rdiv, ps_d)

        # ---- output projection + residual + LayerNorm ----
        # O-proj chains front-load their kt=0..6 steps (probt for dt<7 is
        # long drained); the dt=7 den drain interleaves, and only the final
        # kt=7 step of each chain waits on probt(7).
        if not ln_plain:
            nc.gpsimd.dma_start(out=gamma_sb[:], in_=bcast_dram(hh["gamma"]))
            nc.gpsimd.dma_start(out=beta_sb[:], in_=bcast_dram(hh["beta"]))
        with tc.tile_pool(name="psO", bufs=3, space="PSUM") as psO:
            o_state = {}

            def o_front(it):
                xq_t = xqp.tile([128, 1024], F32, tag="xq",
                                name=f"xq{it}")
                nc.sync.dma_start(out=xq_t[:],
                                  in_=xq_ap[it * 128:(it + 1) * 128, :])
                ps_r = psO.tile([128, 2, 512], F32, tag="oo",
                                name=f"psr{it}")
                for mh in range(2):
                    for kt in range(7):
                        nc.tensor.matmul(
                            ps_r[:, mh, :],
                            probt[:, kt, it * 128:(it + 1) * 128],
                            wo_sb[:, kt, mh * 512:(mh + 1) * 512],
                            start=(kt == 0), stop=False)
                o_state[it] = (ps_r, xq_t)

            o_front(0)
            o_front(1)
            o_front(2)
            # drain dt=7's normalization using a psO bank
            d7, pvsb7, rdiv7, _ = den_prev
            ps_d7 = psO.tile([128, 512], F32, tag="ddt", bufs=1)
            den_prev = (d7, pvsb7, rdiv7, ps_d7)
            for step in range(4):
                den_drain(step)

            for it in range(IT):
                if it == 1:
                    o_front(3)
                ps_r, xq_t = o_state.pop(it)
                for mh in range(2):
                    nc.tensor.matmul(
                        ps_r[:, mh, :],
                        probt[:, 7, it * 128:(it + 1) * 128],
                        wo_sb[:, 7, mh * 512:(mh + 1) * 512],
                        start=False, stop=True)
                h_sb = lnp.tile([128, 1024], F32, tag="ln")
                nc.vector.tensor_add(h_sb[:],
                                     ps_r.rearrange("p a b -> p (a b)"),
                                     xq_t[:])
                stats = statp.tile([128, 2, 6], F32)
                nc.vector.bn_stats(stats[:, 0, :], h_sb[:, 0:512])
                nc.vector.bn_stats(stats[:, 1, :], h_sb[:, 512:1024])
                mv = statp.tile([128, 2], F32)
                nc.vector.bn_aggr(mv[:], stats[:])
                std = statp.tile([128, 1], F32)
                nc.scalar.activation(std[:], mv[:, 1:2], Sqrt,
                                     bias=eps_sb[:], scale=1.0)
                rstd = statp.tile([128, 1], F32)
                nc.vector.reciprocal(rstd[:], std[:])
                nmr = statp.tile([128, 1], F32)
                nc.vector.tensor_scalar(
                    nmr[:], rstd[:], mv[:, 0:1], -1.0,
                    op0=mybir.AluOpType.mult, op1=mybir.AluOpType.mult)
                # (h-mu)*rstd on the idle Scalar engine: h*rstd - mu*rstd
                t1 = lnp.tile([128, 1024], F32, tag="ln")
                nc.scalar.activation(
                    t1[:], h_sb[:], mybir.ActivationFunctionType.Identity,
                    bias=nmr[:], scale=rstd[:])
                if ln_plain:
                    out_t = t1
                else:
                    t2 = lnp.tile([128, 1024], F32, tag="ln")
                    nc.vector.tensor_mul(t2[:], t1[:], gamma_sb[:])
                    out_t = lnp.tile([128, 1024], F32, tag="ln")
                    nc.vector.tensor_add(out_t[:], t2[:], beta_sb[:])
                nc.sync.dma_start(y_ap[it * 128:(it + 1) * 128, :], out_t[:])


def build_module(split=True, masked=False, ln_plain=False):
    nc = bass.Bass("TRN2", target_bir_lowering=False, debug=False,
                   num_devices=N_CORES)
    hh = {
        "xt": nc.dram_tensor("xt", [D, L], BF, kind="ExternalInput"),
        "xq": nc.dram_tensor("xq", [Q, D], F32, kind="ExternalInput"),
        "wq": nc.dram_tensor("wq", [D, D], BF, kind="ExternalInput"),
        "wk": nc.dram_tensor("wk", [D, D], BF, kind="ExternalInput"),
        "wv": nc.dram_tensor("wv", [D, D], BF, kind="ExternalInput"),
        "wo": nc.dram_tensor("wo", [D, D], BF, kind="ExternalInput"),
        "bias": nc.dram_tensor("bias", [16, 128], F32, kind="ExternalInput"),
        "gamma": nc.dram_tensor("gamma", [D], BF, kind="ExternalInput"),
        "beta": nc.dram_tensor("beta", [D], BF, kind="ExternalInput"),
        "y": nc.dram_tensor("y", [Q, D], F32, kind="ExternalOutput"),
    }
    with tile.TileContext(nc) as tc:
        _emit(nc, tc, hh, masked, ln_plain)
    if split:
        _split_waits(nc, 1)
    return nc


_CACHE = {}
_LN_PLAIN = False


def get_module(masked=False):
    key = ("nc", masked, _LN_PLAIN)
    if key not in _CACHE:
        _CACHE[key] = build_module(masked=masked, ln_plain=_LN_PLAIN)
    return _CACHE[key]


def prep_inputs(x, mask, w_q, w_k, w_v, w_o, ln_gamma, ln_beta):
    x = np.asarray(x, dtype=np.float32)
    mask = np.asarray(mask)
    shared = {
        "wq": np.ascontiguousarray(
            np.asarray(w_q, np.float32).transpose(1, 0, 2).reshape(D, D)
        ).astype(BF16),
        "wk": np.ascontiguousarray(
            np.asarray(w_k, np.float32).transpose(1, 0, 2).reshape(D, D)
        ).astype(BF16),
        "wv": np.ascontiguousarray(
            np.asarray(w_v, np.float32).transpose(1, 0, 2).reshape(D, D)
        ).astype(BF16),
        "wo": np.asarray(w_o, np.float32).reshape(D, D).astype(BF16),
        "gamma": np.asarray(ln_gamma, np.float32).astype(BF16),
        "beta": np.asarray(ln_beta, np.float32).astype(BF16),
    }
    in_maps = []
    for c in range(N_CORES):
        b, q0 = c // 4, (c % 4) * Q
        perm = np.r_[q0:L, 0:q0]
        xb = x[b][perm]                       # rotated: q-chunk first
        m = {
            "xt": np.ascontiguousarray(xb.T).astype(BF16),
            "xq": np.ascontiguousarray(x[b, q0:q0 + Q, :]),
            "bias": np.where(mask[b][perm], 0.0, -1e9).astype(
                np.float32).reshape(16, 128),
        }
        m.update(shared)
        in_maps.append(m)
    masked = not bool(mask.all())
    global _LN_PLAIN
    _LN_PLAIN = bool(np.all(np.asarray(ln_gamma) == 1.0)
                     and np.all(np.asarray(ln_beta) == 0.0))
    return in_maps, masked


def assemble(results):
    out = np.empty((B, L, D), dtype=np.float32)
    for c in range(N_CORES):
        b, q0 = c // 4, (c % 4) * Q
        out[b, q0:q0 + Q, :] = results[c]["y"]
    return out


def run(in_maps, masked=False, **kwargs):
    nc = get_module(masked)
    return bass_utils.run_bass_kernel_spmd(
        nc, in_maps, core_ids=list(range(N_CORES)), **kwargs)


def kernel(x, mask, w_q, w_k, w_v, w_o, ln_gamma, ln_beta):
    in_maps, masked = prep_inputs(x, mask, w_q, w_k, w_v, w_o,
                                  ln_gamma, ln_beta)
    res = run(in_maps, masked)
    return assemble(res.results)


# revision 74
# speedup vs baseline: 1.0662x; 1.0662x over previous
"""Trainium2 Bass kernel: fused multi-head attention block (projections +
softmax attention + output projection + residual + LayerNorm).

Sharding: 8 cores = 2 batches x 4 query-chunks of 512. Each core computes
K/V for its whole batch (replicated within the 4-core batch group), Q only
for its 512-query chunk, full attention for that chunk over all 16 heads,
the output projection, residual add and LayerNorm. No collectives.

All cores run the same program; per-core inputs are pre-sliced on the host
with the key/value token order ROTATED so the core's query chunk occupies
rows 0..511 (attention is permutation-invariant over keys, and the key
padding mask is rotated identically).

The d'-tile loop is software-pipelined at key-tile granularity: each jj
step emits logits(dt), two K-projection chain steps for dt+1, and PV
chain steps for dt (consuming exp output just-in-time), so the PE stays
gapless while the Scalar engine's exp stream drains the logits PSUM.

Device-side layouts (per core):
  xt   [1024, 2048] bf16  x[b] transposed (feature-major), rotated
  xq   [512, 1024]  f32   query-chunk rows of x[b] (residual input)
  wq/wk/wv [1024, 1024] bf16  [c, h*64] (head-minor)
  wo   [1024, 1024] bf16  [(h*64+d), m]
  bias [16, 128]    f32   additive key mask bias per key tile/partition
  gamma/beta [1024] bf16
Output: y [512, 1024] f32.
"""

import contextlib

import numpy as np
import ml_dtypes

import concourse.bass as bass
import concourse.tile as tile
from concourse import mybir
from concourse import bass_utils

BF16 = ml_dtypes.bfloat16
N_CORES = 8
B, L, D, H, DH = 2, 2048, 1024, 16, 64
Q = L // 4          # queries per core
CT = D // 128       # contraction tiles over features
JT = L // 128       # key tiles
IT = Q // 128       # query tiles
LN_EPS = 1e-5

F32 = mybir.dt.float32
BF = mybir.dt.bfloat16


def _split_waits(nc, maxw=1):
    """This walrus build rejects instructions with more than one sync wait;
    split excess waits into preceding NOPs on the same engine."""
    ctr = 0
    for fn in nc.m.functions:
        for bb in fn.blocks:
            new_insts = []
            for inst in bb.instructions:
                si = inst.sync_info
                if si is not None and len(si.on_wait) > maxw:
                    waits = list(si.on_wait)
                    excess, keep = waits[:-maxw], waits[-maxw:]
                    for i in range(0, len(excess), maxw):
                        ctr += 1
                        new_insts.append(mybir.InstNoOp(
                            name=f"waitsplit_nop_{ctr}",
                            engine=inst.engine,
                            sync_info=mybir.SyncInfo(
                                on_wait=excess[i:i + maxw], on_update=[]),
                            text_hint="waitsplit",
                        ))
                    si.on_wait = keep
                new_insts.append(inst)
            bb.instructions = new_insts
    return ctr


def _emit(nc, tc, hh, masked, ln_plain=False):
    Exp = mybir.ActivationFunctionType.Exp
    Sqrt = mybir.ActivationFunctionType.Sqrt

    xt_ap = hh["xt"].ap().rearrange("(t p) l -> p t l", p=128)      # [128,8,2048]
    wq_ap = hh["wq"].ap().rearrange("(t p) d -> p t d", p=128)
    wk_ap = hh["wk"].ap().rearrange("(t p) d -> p t d", p=128)
    wv_ap = hh["wv"].ap().rearrange("(t p) d -> p t d", p=128)
    wo_ap = hh["wo"].ap().rearrange("(t p) d -> p t d", p=128)
    bias_ap = hh["bias"].ap().rearrange("a b -> b a")               # [128,16]
    xq_ap = hh["xq"].ap()
    y_ap = hh["y"].ap()

    def bcast_dram(h1d, parts=128):
        a = h1d.ap()
        return bass.AP(tensor=a.tensor, offset=a.offset,
                       ap=[[0, parts]] + list(a.ap))

    with contextlib.ExitStack() as ctx:
        const = ctx.enter_context(tc.tile_pool(name="const", bufs=1))
        wpool = ctx.enter_context(tc.tile_pool(name="wpool", bufs=2))
        xtp = ctx.enter_context(tc.tile_pool(name="xtp", bufs=2))
        expp = ctx.enter_context(tc.tile_pool(name="expp", bufs=2))
        ktp = ctx.enter_context(tc.tile_pool(name="ktp", bufs=2))
        vp = ctx.enter_context(tc.tile_pool(name="vp", bufs=1))
        qtp = ctx.enter_context(tc.tile_pool(name="qtp", bufs=1))
        ptp = ctx.enter_context(tc.tile_pool(name="ptp", bufs=1))
        npool = ctx.enter_context(tc.tile_pool(name="npool", bufs=3))
        xqp = ctx.enter_context(tc.tile_pool(name="xqp", bufs=4))
        lnp = ctx.enter_context(tc.tile_pool(name="lnp", bufs=3))
        statp = ctx.enter_context(tc.tile_pool(name="statp", bufs=4))

        # ---- constants / small loads ----
        eps_sb = const.tile([128, 1], F32)
        nc.vector.memset(eps_sb[:], LN_EPS)
        ones64 = const.tile([33, 64], BF)
        nc.vector.memset(ones64[:], 1.0)
        den2_bufs = []
        for i in range(2):
            d2 = const.tile([33, 512], F32, name=f"den2buf{i}")
            nc.vector.memset(d2[:], 1.0)
            den2_bufs.append(d2)
        bias_sb = const.tile([128, 16], F32)
        nc.gpsimd.dma_start(out=bias_sb[:], in_=bias_ap)
        gamma_sb = const.tile([128, 1024], BF)
        beta_sb = const.tile([128, 1024], BF)

        # ---- big SBUF tensors ----
        v_all = vp.tile([128, JT, H, DH + 1], BF)  # V by key tile, +ones col
        qt_all = qtp.tile([128, 8, Q], BF)         # Q^T for the chunk
        probt = ptp.tile([128, 8, Q], BF)          # normalized P^T stacked

        nc.vector.memset(v_all[:, :, :, DH:DH + 1], 1.0)

        # x is the gate for the first projections: split it across the SP
        # and ACT DMA queues (1MB halves land first), and push the weights
        # to the DVE/POOL queues so they stream in parallel.
        # startup is DMA-bandwidth-bound: wv halves land first on the two
        # HWDGE queues, then x arrives in key-column chunks matched to the
        # V-chain consumption order (chain lt needs x columns lt*128..),
        # so the PE starts ~17us instead of waiting for all 6MB.
        xt0 = xtp.tile([128, 4, 2048], BF, tag="xt")
        xt1 = xtp.tile([128, 4, 2048], BF, tag="xt")
        wv_sb = wpool.tile([128, 8, 1024], BF, tag="w")
        nc.sync.dma_start(out=wv_sb[:, :, 0:512], in_=wv_ap[:, :, 0:512])
        nc.scalar.dma_start(out=wv_sb[:, :, 512:1024],
                            in_=wv_ap[:, :, 512:1024])
        for c in range(4):
            lo, hi = c * 512, (c + 1) * 512
            nc.sync.dma_start(out=xt0[:, :, lo:hi],
                              in_=xt_ap[:, 0:4, lo:hi])
            nc.scalar.dma_start(out=xt1[:, :, lo:hi],
                                in_=xt_ap[:, 4:8, lo:hi])
        wq_sb = wpool.tile([128, 8, 1024], BF, tag="w")
        nc.gpsimd.dma_start(out=wq_sb[:], in_=wq_ap)

        def xt_sl(ct, lo, size):
            t = xt0 if ct < 4 else xt1
            return t[:, ct % 4, lo:lo + size]

        kt_tiles = {}

        with tc.tile_pool(name="psA", bufs=3, space="PSUM") as psA:
            # ---- V projection: [token 128][h*64] ----
            for lt in range(JT):
                ps = psA.tile([128, 2, 512], F32, tag="aa")
                for nt in range(2):
                    for ct in range(CT):
                        nc.tensor.matmul(
                            ps[:, nt, :], xt_sl(ct, lt * 128, 128),
                            wv_sb[:, ct, nt * 512:(nt + 1) * 512],
                            start=(ct == 0), stop=(ct == CT - 1))
                nc.vector.tensor_copy(
                    v_all[:, lt, :, 0:DH],
                    ps.rearrange("p n (h d) -> p (n h) d", h=8))

            wk_sb = wpool.tile([128, 8, 1024], BF, tag="w")
            nc.gpsimd.dma_start(out=wk_sb[:], in_=wk_ap)

            # ---- K^T projection for dt=0, first half (so its eviction
            # lands during Q-proj and logits(0) never waits) ----
            kt0 = ktp.tile([128, 2048], BF, tag="kt")
            kt_tiles[0] = kt0

            def k0_half(jp):
                ps = psA.tile([128, 2, 512], F32, tag="aa",
                              name=f"k0p{jp}")
                for j4 in range(2):
                    for ct in range(CT):
                        nc.tensor.matmul(
                            ps[:, j4, :],
                            wk_sb[:, ct, 0:128],
                            xt_sl(ct, jp * 1024 + j4 * 512, 512),
                            start=(ct == 0), stop=(ct == CT - 1))
                nc.vector.tensor_copy(
                    kt0[:, jp * 1024:(jp + 1) * 1024], ps[:])

            # ---- Q^T projection: [d' 128][i 512] ----
            for dtp in range(4):
                ps = psA.tile([128, 2, 512], F32, tag="aa")
                for half in range(2):
                    dt = 2 * dtp + half
                    for ct in range(CT):
                        nc.tensor.matmul(
                            ps[:, half, :],
                            wq_sb[:, ct, dt * 128:(dt + 1) * 128],
                            xt_sl(ct, 0, Q),
                            start=(ct == 0), stop=(ct == CT - 1))
                nc.vector.tensor_copy(qt_all[:, 2 * dtp:2 * dtp + 2, :], ps[:])

            wo_sb = wpool.tile([128, 8, 1024], BF, tag="w")
            nc.gpsimd.dma_start(out=wo_sb[:], in_=wo_ap)

            k0_half(0)
            k0_half(1)

        # ---- attention: software-pipelined over (dt, jj) ----
        # per jj step: logits(dt, jj) x2hb, two K-proj(dt+1) chain steps,
        # PV(dt, jj-2) x2hb; exp on Scalar drains logits PSUM. The
        # denominator path is decoupled from every PSUM pool the PE loops
        # on: PV PSUM is staged to SBUF right away (freeing psP), the slow
        # fixed-cost Vector reciprocal runs SBUF-only, and the PE broadcast
        # + normalize mul for dt are emitted inside dt+1's jj stream, by
        # which point the reciprocal is long done. K evictions go on
        # Scalar so Vector's reciprocals never delay them.
        with tc.tile_pool(name="psL", bufs=2, space="PSUM") as psL, \
             tc.tile_pool(name="psK", bufs=1, space="PSUM") as psK, \
             tc.tile_pool(name="psP", bufs=2, space="PSUM") as psP, \
             tc.tile_pool(name="psD", bufs=1, space="PSUM") as psD:
            den_prev = None
            for dt in range(8):
                kt_cur = kt_tiles.pop(dt)
                if dt < 7:
                    kt_nxt = ktp.tile([128, 2048], BF, tag="kt")
                    kt_tiles[dt + 1] = kt_nxt
                expt_h = [expp.tile([128, JT // 2, 2, 512], BF, tag="e",
                                    name=f"expt{dt}_{h}") for h in range(2)]

                def expt(jj):
                    return expt_h[jj // (JT // 2)][:, jj % (JT // 2), :, :]
                pv_ps = [psP.tile([DH + 1, 512], F32, tag="pp",
                                  name=f"pv{dt}_{hb}") for hb in range(2)]
                kstate = {}

                def kstep(s):
                    # chain c = s//8 covers keys [c*512, (c+1)*512), ct = s%8
                    c, ct = s // 8, s % 8
                    if ct == 0:
                        kstate[c] = psK.tile([128, 512], F32, tag="kk",
                                             name=f"k{dt}_{c}")
                    nc.tensor.matmul(
                        kstate[c][:],
                        wk_sb[:, ct, (dt + 1) * 128:(dt + 2) * 128],
                        xt_sl(ct, c * 512, 512),
                        start=(ct == 0), stop=(ct == CT - 1))
                    if ct == CT - 1:
                        # chains whose eviction gates the next psK alloc
                        # tightly go on Scalar; the slack ones on Vector
                        # (whose queue may be behind a 3.3us reciprocal).
                        if c % 2 == 0:
                            nc.scalar.copy(
                                kt_nxt[:, c * 512:(c + 1) * 512],
                                kstate.pop(c))
                        else:
                            nc.vector.tensor_copy(
                                kt_nxt[:, c * 512:(c + 1) * 512],
                                kstate.pop(c))

                def pvstep(jj):
                    for hb in range(2):
                        nc.tensor.matmul(
                            pv_ps[hb][:], v_all[:, jj, 2 * dt + hb, 0:DH + 1],
                            expt(jj)[:, hb, :],
                            start=(jj == 0), stop=(jj == JT - 1))

                def den_drain(step):
                    # finish dt-1's normalization: PE broadcasts 1/den, then
                    # Vector scales the staged P^T into probt.
                    if den_prev is None:
                        return
                    d, pvsb, rdiv, ps_d = den_prev
                    if step == 0:
                        nc.tensor.matmul(ps_d[0:64, :], ones64[0:1, :],
                                         rdiv[0], start=True, stop=True)
                    elif step == 1:
                        nc.tensor.matmul(ps_d[64:128, :], ones64[32:33, :],
                                         rdiv[1], start=True, stop=True)
                    elif step == 2:
                        nc.vector.tensor_mul(probt[0:64, d, :],
                                             pvsb[0][0:DH, :], ps_d[0:64, :])
                    else:
                        nc.vector.tensor_mul(probt[64:128, d, :],
                                             pvsb[1][0:DH, :],
                                             ps_d[64:128, :])

                for jj in range(JT):
                    if jj >= 2:
                        pvstep(jj - 2)
                    # K steps before the logits pair: their 128-col
                    # LDWEIGHTS hides behind full-length PV/K matmuls, and
                    # the logits pair stays contiguous with its exp so the
                    # psL drain pacing is unchanged.
                    if dt < 7:
                        kstep(2 * jj)
                        kstep(2 * jj + 1)
                    psq = psL.tile([128, 2, 512], F32, tag="ll")
                    for hb in range(2):
                        nc.tensor.matmul(
                            psq[:, hb, :],
                            kt_cur[hb * 64:hb * 64 + 64,
                                   jj * 128:(jj + 1) * 128],
                            qt_all[hb * 64:hb * 64 + 64, dt, :],
                            start=True, stop=True)
                    if masked:
                        for hb in range(2):
                            nc.scalar.activation(
                                expt(jj)[:, hb, :], psq[:, hb, :], Exp,
                                bias=bias_sb[:, jj:jj + 1], scale=1.0 / 8.0)
                    else:
                        nc.scalar.activation(
                            expt(jj).rearrange("p a b -> p (a b)"),
                            psq.rearrange("p a b -> p (a b)"), Exp,
                            bias=0.0, scale=1.0 / 8.0)
                    if jj in (6, 8, 10, 12):
                        den_drain((jj - 6) // 2)
                pvstep(JT - 2)
                pvstep(JT - 1)

                # stage PV PSUM to SBUF (frees psP) and take the reciprocal
                # off-path; the broadcast+mul happen during dt+1. The DVE
                # reciprocal has a ~3.3us fixed cost, so both heads'
                # denominators share one [1,1024] instruction.
                pvsb = []
                den2 = den2_bufs[dt % 2]
                for hb in range(2):
                    t = npool.tile([DH + 1, 512], F32, tag="pv",
                                   name=f"pvsb{dt}_{hb}")
                    nc.vector.tensor_copy(t[:], pv_ps[hb][:])
                    pvsb.append(t)
                    nc.vector.tensor_copy(den2[32 * hb:32 * hb + 1, :],
                                          t[DH:DH + 1, :])
                # reciprocal cost scales with free size only, so one [33,512]
                # instruction covers both heads (rows 1..31 are don't-care).
                rdivw = npool.tile([33, 512], BF, tag="n",
                                   name=f"rdivw{dt}")
                with nc.allow_low_precision(
                        reason="bf16 1/den: per-(h,q) scale, ~0.2%"):
                    nc.vector.reciprocal(rdivw[:], den2[:])
                rdiv = [rdivw[0:1, :], rdivw[32:33, :]]
                if dt < 7:
                    ps_d = psD.tile([128, 512], F32, tag="dd")
                else:
                    ps_d = None  # dt=7 drains in the tail from the psO pool
                den_prev = (dt, pvsb, rdiv, ps_d)

        # ---- output projection + residual + LayerNorm ----
        # O-proj chains front-load their kt=0..6 steps (probt for dt<7 is
        # long drained); the dt=7 den drain interleaves, and only the final
        # kt=7 step of each chain waits on probt(7).
        if not ln_plain:
            nc.gpsimd.dma_start(out=gamma_sb[:], in_=bcast_dram(hh["gamma"]))
            nc.gpsimd.dma_start(out=beta_sb[:], in_=bcast_dram(hh["beta"]))
        with tc.tile_pool(name="psO", bufs=3, space="PSUM") as psO:
            o_state = {}

            def o_front(it):
                xq_t = xqp.tile([128, 1024], F32, tag="xq",
                                name=f"xq{it}")
                nc.sync.dma_start(out=xq_t[:],
                                  in_=xq_ap[it * 128:(it + 1) * 128, :])
                ps_r = psO.tile([128, 2, 512], F32, tag="oo",
                                name=f"psr{it}")
                for mh in range(2):
                    for kt in range(7):
                        nc.tensor.matmul(
                            ps_r[:, mh, :],
                            probt[:, kt, it * 128:(it + 1) * 128],
                            wo_sb[:, kt, mh * 512:(mh + 1) * 512],
                            start=(kt == 0), stop=False)
                o_state[it] = (ps_r, xq_t)

            o_front(0)
            o_front(1)
            o_front(2)
            # drain dt=7's normalization using a psO bank
            d7, pvsb7, rdiv7, _ = den_prev
            ps_d7 = psO.tile([128, 512], F32, tag="ddt", bufs=1)
            den_prev = (d7, pvsb7, rdiv7, ps_d7)
            for step in range(4):
                den_drain(step)

            for it in range(IT):
                if it == 1:
                    o_front(3)
                ps_r, xq_t = o_state.pop(it)
                for mh in range(2):
                    nc.tensor.matmul(
                        ps_r[:, mh, :],
                        probt[:, 7, it * 128:(it + 1) * 128],
                        wo_sb[:, 7, mh * 512:(mh + 1) * 512],
                        start=False, stop=True)
                h_sb = lnp.tile([128, 1024], F32, tag="ln")
                nc.vector.tensor_add(h_sb[:],
                                     ps_r.rearrange("p a b -> p (a b)"),
                                     xq_t[:])
                stats = statp.tile([128, 2, 6], F32)
                nc.vector.bn_stats(stats[:, 0, :], h_sb[:, 0:512])
                nc.vector.bn_stats(stats[:, 1, :], h_sb[:, 512:1024])
                mv = statp.tile([128, 2], F32)
                nc.vector.bn_aggr(mv[:], stats[:])
                std = statp.tile([128, 1], F32)
                nc.scalar.activation(std[:], mv[:, 1:2], Sqrt,
                                     bias=eps_sb[:], scale=1.0)
                rstd = statp.tile([128, 1], F32)
                nc.vector.reciprocal(rstd[:], std[:])
                nmr = statp.tile([128, 1], F32)
                nc.vector.tensor_scalar(
                    nmr[:], rstd[:], mv[:, 0:1], -1.0,
                    op0=mybir.AluOpType.mult, op1=mybir.AluOpType.mult)
                # (h-mu)*rstd on the idle Scalar engine: h*rstd - mu*rstd
                t1 = lnp.tile([128, 1024], F32, tag="ln")
                nc.scalar.activation(
                    t1[:], h_sb[:], mybir.ActivationFunctionType.Identity,
                    bias=nmr[:], scale=rstd[:])
                if ln_plain:
                    out_t = t1
                else:
                    t2 = lnp.tile([128, 1024], F32, tag="ln")
                    nc.vector.tensor_mul(t2[:], t1[:], gamma_sb[:])
                    out_t = lnp.tile([128, 1024], F32, tag="ln")
                    nc.vector.tensor_add(out_t[:], t2[:], beta_sb[:])
                nc.sync.dma_start(y_ap[it * 128:(it + 1) * 128, :], out_t[:])


def build_module(split=True, masked=False, ln_plain=False):
    nc = bass.Bass("TRN2", target_bir_lowering=False, debug=False,
                   num_devices=N_CORES)
    hh = {
        "xt": nc.dram_tensor("xt", [D, L], BF, kind="ExternalInput"),
        "xq": nc.dram_tensor("xq", [Q, D], F32, kind="ExternalInput"),
        "wq": nc.dram_tensor("wq", [D, D], BF, kind="ExternalInput"),
        "wk": nc.dram_tensor("wk", [D, D], BF, kind="ExternalInput"),
        "wv": nc.dram_tensor("wv", [D, D], BF, kind="ExternalInput"),
        "wo": nc.dram_tensor("wo", [D, D], BF, kind="ExternalInput"),
        "bias": nc.dram_tensor("bias", [16, 128], F32, kind="ExternalInput"),
        "gamma": nc.dram_tensor("gamma", [D], BF, kind="ExternalInput"),
        "beta": nc.dram_tensor("beta", [D], BF, kind="ExternalInput"),
        "y": nc.dram_tensor("y", [Q, D], F32, kind="ExternalOutput"),
    }
    with tile.TileContext(nc) as tc:
        _emit(nc, tc, hh, masked, ln_plain)
    if split:
        _split_waits(nc, 1)
    return nc


_CACHE = {}
_LN_PLAIN = False


def get_module(masked=False):
    key = ("nc", masked, _LN_PLAIN)
    if key not in _CACHE:
        _CACHE[key] = build_module(masked=masked, ln_plain=_LN_PLAIN)
    return _CACHE[key]


def prep_inputs(x, mask, w_q, w_k, w_v, w_o, ln_gamma, ln_beta):
    x = np.asarray(x, dtype=np.float32)
    mask = np.asarray(mask)
    shared = {
        "wq": np.ascontiguousarray(
            np.asarray(w_q, np.float32).transpose(1, 0, 2).reshape(D, D)
        ).astype(BF16),
        "wk": np.ascontiguousarray(
            np.asarray(w_k, np.float32).transpose(1, 0, 2).reshape(D, D)
        ).astype(BF16),
        "wv": np.ascontiguousarray(
            np.asarray(w_v, np.float32).transpose(1, 0, 2).reshape(D, D)
        ).astype(BF16),
        "wo": np.asarray(w_o, np.float32).reshape(D, D).astype(BF16),
        "gamma": np.asarray(ln_gamma, np.float32).astype(BF16),
        "beta": np.asarray(ln_beta, np.float32).astype(BF16),
    }
    in_maps = []
    for c in range(N_CORES):
        b, q0 = c // 4, (c % 4) * Q
        perm = np.r_[q0:L, 0:q0]
        xb = x[b][perm]                       # rotated: q-chunk first
        m = {
            "xt": np.ascontiguousarray(xb.T).astype(BF16),
            "xq": np.ascontiguousarray(x[b, q0:q0 + Q, :]),
            "bias": np.where(mask[b][perm], 0.0, -1e9).astype(
                np.float32).reshape(16, 128),
        }
        m.update(shared)
        in_maps.append(m)
    masked = not bool(mask.all())
    global _LN_PLAIN
    _LN_PLAIN = bool(np.all(np.asarray(ln_gamma) == 1.0)
                     and np.all(np.asarray(ln_beta) == 0.0))
    return in_maps, masked


def assemble(results):
    out = np.empty((B, L, D), dtype=np.float32)
    for c in range(N_CORES):
        b, q0 = c // 4, (c % 4) * Q
        out[b, q0:q0 + Q, :] = results[c]["y"]
    return out


def run(in_maps, masked=False, **kwargs):
    nc = get_module(masked)
    return bass_utils.run_bass_kernel_spmd(
        nc, in_maps, core_ids=list(range(N_CORES)), **kwargs)


def kernel(x, mask, w_q, w_k, w_v, w_o, ln_gamma, ln_beta):
    in_maps, masked = prep_inputs(x, mask, w_q, w_k, w_v, w_o,
                                  ln_gamma, ln_beta)
    res = run(in_maps, masked)
    return assemble(res.results)


# revision 76
# speedup vs baseline: 1.0890x; 1.0214x over previous
"""Trainium2 Bass kernel: fused multi-head attention block (projections +
softmax attention + output projection + residual + LayerNorm).

Sharding: 8 cores = 2 batches x 4 query-chunks of 512. Each core computes
K/V for its whole batch (replicated within the 4-core batch group), Q only
for its 512-query chunk, full attention for that chunk over all 16 heads,
the output projection, residual add and LayerNorm. No collectives.

All cores run the same program; per-core inputs are pre-sliced on the host
with the key/value token order ROTATED so the core's query chunk occupies
rows 0..511 (attention is permutation-invariant over keys, and the key
padding mask is rotated identically).

The d'-tile loop is software-pipelined at key-tile granularity: each jj
step emits logits(dt), two K-projection chain steps for dt+1, and PV
chain steps for dt (consuming exp output just-in-time), so the PE stays
gapless while the Scalar engine's exp stream drains the logits PSUM.

Device-side layouts (per core):
  xt   [1024, 2048] bf16  x[b] transposed (feature-major), rotated
  xq   [512, 1024]  f32   query-chunk rows of x[b] (residual input)
  wq/wk/wv [1024, 1024] bf16  [c, h*64] (head-minor)
  wo   [1024, 1024] bf16  [(h*64+d), m]
  bias [16, 128]    f32   additive key mask bias per key tile/partition
  gamma/beta [1024] bf16
Output: y [512, 1024] f32.
"""

import contextlib

import numpy as np
import ml_dtypes

import concourse.bass as bass
import concourse.tile as tile
from concourse import mybir
from concourse import bass_utils

BF16 = ml_dtypes.bfloat16
N_CORES = 8
B, L, D, H, DH = 2, 2048, 1024, 16, 64
Q = L // 4          # queries per core
CT = D // 128       # contraction tiles over features
JT = L // 128       # key tiles
IT = Q // 128       # query tiles
LN_EPS = 1e-5

F32 = mybir.dt.float32
BF = mybir.dt.bfloat16


def _split_waits(nc, maxw=1):
    """This walrus build rejects instructions with more than one sync wait;
    split excess waits into preceding NOPs on the same engine."""
    ctr = 0
    for fn in nc.m.functions:
        for bb in fn.blocks:
            new_insts = []
            for inst in bb.instructions:
                si = inst.sync_info
                if si is not None and len(si.on_wait) > maxw:
                    waits = list(si.on_wait)
                    excess, keep = waits[:-maxw], waits[-maxw:]
                    for i in range(0, len(excess), maxw):
                        ctr += 1
                        new_insts.append(mybir.InstNoOp(
                            name=f"waitsplit_nop_{ctr}",
                            engine=inst.engine,
                            sync_info=mybir.SyncInfo(
                                on_wait=excess[i:i + maxw], on_update=[]),
                            text_hint="waitsplit",
                        ))
                    si.on_wait = keep
                new_insts.append(inst)
            bb.instructions = new_insts
    return ctr


def _emit(nc, tc, hh, masked, ln_plain=False):
    Exp = mybir.ActivationFunctionType.Exp
    Sqrt = mybir.ActivationFunctionType.Sqrt

    xt_ap = hh["xt"].ap().rearrange("(t p) l -> p t l", p=128)      # [128,8,2048]
    wq_ap = hh["wq"].ap().rearrange("(t p) d -> p t d", p=128)
    wk_ap = hh["wk"].ap().rearrange("(t p) d -> p t d", p=128)
    wv_ap = hh["wv"].ap().rearrange("(t p) d -> p t d", p=128)
    wo_ap = hh["wo"].ap().rearrange("(t p) d -> p t d", p=128)
    bias_ap = hh["bias"].ap().rearrange("a b -> b a")               # [128,16]
    xq_ap = hh["xq"].ap()
    y_ap = hh["y"].ap()

    def bcast_dram(h1d, parts=128):
        a = h1d.ap()
        return bass.AP(tensor=a.tensor, offset=a.offset,
                       ap=[[0, parts]] + list(a.ap))

    with contextlib.ExitStack() as ctx:
        const = ctx.enter_context(tc.tile_pool(name="const", bufs=1))
        wpool = ctx.enter_context(tc.tile_pool(name="wpool", bufs=2))
        xtp = ctx.enter_context(tc.tile_pool(name="xtp", bufs=2))
        expp = ctx.enter_context(tc.tile_pool(name="expp", bufs=2))
        ktp = ctx.enter_context(tc.tile_pool(name="ktp", bufs=2))
        vp = ctx.enter_context(tc.tile_pool(name="vp", bufs=1))
        qtp = ctx.enter_context(tc.tile_pool(name="qtp", bufs=1))
        ptp = ctx.enter_context(tc.tile_pool(name="ptp", bufs=1))
        npool = ctx.enter_context(tc.tile_pool(name="npool", bufs=3))
        xqp = ctx.enter_context(tc.tile_pool(name="xqp", bufs=4))
        lnp = ctx.enter_context(tc.tile_pool(name="lnp", bufs=3))
        statp = ctx.enter_context(tc.tile_pool(name="statp", bufs=4))

        # ---- constants / small loads ----
        eps_sb = const.tile([128, 1], F32)
        nc.vector.memset(eps_sb[:], LN_EPS)
        ones64 = const.tile([33, 64], BF)
        nc.vector.memset(ones64[:], 1.0)
        den2_bufs = []
        for i in range(2):
            d2 = const.tile([33, 512], F32, name=f"den2buf{i}")
            nc.vector.memset(d2[:], 1.0)
            den2_bufs.append(d2)
        bias_sb = const.tile([128, 16], F32)
        nc.gpsimd.dma_start(out=bias_sb[:], in_=bias_ap)
        gamma_sb = const.tile([128, 1024], BF)
        beta_sb = const.tile([128, 1024], BF)

        # ---- big SBUF tensors ----
        v_all = vp.tile([128, JT, H, DH + 1], BF)  # V by key tile, +ones col
        qt_all = qtp.tile([128, 8, Q], BF)         # Q^T for the chunk
        probt = ptp.tile([128, 8, Q], BF)          # normalized P^T stacked

        nc.vector.memset(v_all[:, :, :, DH:DH + 1], 1.0)

        # x is the gate for the first projections: split it across the SP
        # and ACT DMA queues (1MB halves land first), and push the weights
        # to the DVE/POOL queues so they stream in parallel.
        # startup is DMA-bandwidth-bound: wv halves land first on the two
        # HWDGE queues, then x arrives in key-column chunks matched to the
        # V-chain consumption order (chain lt needs x columns lt*128..),
        # so the PE starts ~17us instead of waiting for all 6MB.
        xt0 = xtp.tile([128, 4, 2048], BF, tag="xt")
        xt1 = xtp.tile([128, 4, 2048], BF, tag="xt")
        wv_sb = wpool.tile([128, 8, 1024], BF, tag="w")
        nc.sync.dma_start(out=wv_sb[:, :, 0:512], in_=wv_ap[:, :, 0:512])
        nc.scalar.dma_start(out=wv_sb[:, :, 512:1024],
                            in_=wv_ap[:, :, 512:1024])
        for c in range(4):
            lo, hi = c * 512, (c + 1) * 512
            nc.sync.dma_start(out=xt0[:, :, lo:hi],
                              in_=xt_ap[:, 0:4, lo:hi])
            nc.scalar.dma_start(out=xt1[:, :, lo:hi],
                                in_=xt_ap[:, 4:8, lo:hi])
        wq_sb = wpool.tile([128, 8, 1024], BF, tag="w")
        nc.gpsimd.dma_start(out=wq_sb[:], in_=wq_ap)

        def xt_sl(ct, lo, size):
            t = xt0 if ct < 4 else xt1
            return t[:, ct % 4, lo:lo + size]

        kt_tiles = {}

        with tc.tile_pool(name="psA", bufs=3, space="PSUM") as psA:
            # ---- V projection: [token 128][h*64] ----
            for lt in range(JT):
                ps = psA.tile([128, 2, 512], F32, tag="aa")
                for nt in range(2):
                    for ct in range(CT):
                        nc.tensor.matmul(
                            ps[:, nt, :], xt_sl(ct, lt * 128, 128),
                            wv_sb[:, ct, nt * 512:(nt + 1) * 512],
                            start=(ct == 0), stop=(ct == CT - 1))
                nc.vector.tensor_copy(
                    v_all[:, lt, :, 0:DH],
                    ps.rearrange("p n (h d) -> p (n h) d", h=8))

            wk_sb = wpool.tile([128, 8, 1024], BF, tag="w")
            nc.gpsimd.dma_start(out=wk_sb[:], in_=wk_ap)

            # ---- K^T projection for dt=0, first half (so its eviction
            # lands during Q-proj and logits(0) never waits) ----
            kt0 = ktp.tile([128, 2048], BF, tag="kt")
            kt_tiles[0] = kt0

            def k0_half(jp):
                ps = psA.tile([128, 2, 512], F32, tag="aa",
                              name=f"k0p{jp}")
                for j4 in range(2):
                    for ct in range(CT):
                        nc.tensor.matmul(
                            ps[:, j4, :],
                            wk_sb[:, ct, 0:128],
                            xt_sl(ct, jp * 1024 + j4 * 512, 512),
                            start=(ct == 0), stop=(ct == CT - 1))
                nc.vector.tensor_copy(
                    kt0[:, jp * 1024:(jp + 1) * 1024], ps[:])

            # ---- Q^T projection: [d' 128][i 512] ----
            for dtp in range(4):
                ps = psA.tile([128, 2, 512], F32, tag="aa")
                for half in range(2):
                    dt = 2 * dtp + half
                    for ct in range(CT):
                        nc.tensor.matmul(
                            ps[:, half, :],
                            wq_sb[:, ct, dt * 128:(dt + 1) * 128],
                            xt_sl(ct, 0, Q),
                            start=(ct == 0), stop=(ct == CT - 1))
                nc.vector.tensor_copy(qt_all[:, 2 * dtp:2 * dtp + 2, :], ps[:])

            wo_sb = wpool.tile([128, 8, 1024], BF, tag="w")
            nc.gpsimd.dma_start(out=wo_sb[:], in_=wo_ap)

            k0_half(0)
            k0_half(1)

        # ---- attention: software-pipelined over (dt, jj) ----
        # per jj step: logits(dt, jj) x2hb, two K-proj(dt+1) chain steps,
        # PV(dt, jj-2) x2hb; exp on Scalar drains logits PSUM. The
        # denominator path is decoupled from every PSUM pool the PE loops
        # on: PV PSUM is staged to SBUF right away (freeing psP), the slow
        # fixed-cost Vector reciprocal runs SBUF-only, and the PE broadcast
        # + normalize mul for dt are emitted inside dt+1's jj stream, by
        # which point the reciprocal is long done. K evictions go on
        # Scalar so Vector's reciprocals never delay them.
        with tc.tile_pool(name="psL", bufs=2, space="PSUM") as psL, \
             tc.tile_pool(name="psK", bufs=1, space="PSUM") as psK, \
             tc.tile_pool(name="psP", bufs=2, space="PSUM") as psP, \
             tc.tile_pool(name="psD", bufs=1, space="PSUM") as psD:
            den_prev = None
            for dt in range(8):
                kt_cur = kt_tiles.pop(dt)
                if dt < 7:
                    kt_nxt = ktp.tile([128, 2048], BF, tag="kt")
                    kt_tiles[dt + 1] = kt_nxt
                expt_h = [expp.tile([128, JT // 2, 2, 512], BF, tag="e",
                                    name=f"expt{dt}_{h}") for h in range(2)]

                def expt(jj):
                    return expt_h[jj // (JT // 2)][:, jj % (JT // 2), :, :]
                pv_ps = [psP.tile([DH + 1, 512], F32, tag="pp",
                                  name=f"pv{dt}_{hb}") for hb in range(2)]
                kstate = {}

                def kstep(s):
                    # chain c = s//8 covers keys [c*512, (c+1)*512), ct = s%8
                    c, ct = s // 8, s % 8
                    if ct == 0:
                        kstate[c] = psK.tile([128, 512], F32, tag="kk",
                                             name=f"k{dt}_{c}")
                    nc.tensor.matmul(
                        kstate[c][:],
                        wk_sb[:, ct, (dt + 1) * 128:(dt + 2) * 128],
                        xt_sl(ct, c * 512, 512),
                        start=(ct == 0), stop=(ct == CT - 1))
                    if ct == CT - 1:
                        # all evictions on Vector: with the reciprocal at
                        # the dt end, Vector's queue is clear here, and
                        # keeping them off Scalar keeps the exp stream
                        # (which paces the whole pipeline) dense.
                        nc.vector.tensor_copy(
                            kt_nxt[:, c * 512:(c + 1) * 512],
                            kstate.pop(c))

                def pvstep(jj):
                    for hb in range(2):
                        nc.tensor.matmul(
                            pv_ps[hb][:], v_all[:, jj, 2 * dt + hb, 0:DH + 1],
                            expt(jj)[:, hb, :],
                            start=(jj == 0), stop=(jj == JT - 1))

                def den_drain(step):
                    # finish dt-1's normalization: PE broadcasts 1/den, then
                    # Vector scales the staged P^T into probt.
                    if den_prev is None:
                        return
                    d, pvsb, rdiv, ps_d = den_prev
                    if step == 0:
                        nc.tensor.matmul(ps_d[0:64, :], ones64[0:1, :],
                                         rdiv[0], start=True, stop=True)
                    elif step == 1:
                        nc.tensor.matmul(ps_d[64:128, :], ones64[32:33, :],
                                         rdiv[1], start=True, stop=True)
                    elif step == 2:
                        nc.vector.tensor_mul(probt[0:64, d, :],
                                             pvsb[0][0:DH, :], ps_d[0:64, :])
                    else:
                        nc.vector.tensor_mul(probt[64:128, d, :],
                                             pvsb[1][0:DH, :],
                                             ps_d[64:128, :])

                for jj in range(JT):
                    if jj >= 2:
                        pvstep(jj - 2)
                    psq = psL.tile([128, 2, 512], F32, tag="ll")
                    for hb in range(2):
                        nc.tensor.matmul(
                            psq[:, hb, :],
                            kt_cur[hb * 64:hb * 64 + 64,
                                   jj * 128:(jj + 1) * 128],
                            qt_all[hb * 64:hb * 64 + 64, dt, :],
                            start=True, stop=True)
                    if masked:
                        for hb in range(2):
                            nc.scalar.activation(
                                expt(jj)[:, hb, :], psq[:, hb, :], Exp,
                                bias=bias_sb[:, jj:jj + 1], scale=1.0 / 8.0)
                    else:
                        nc.scalar.activation(
                            expt(jj).rearrange("p a b -> p (a b)"),
                            psq.rearrange("p a b -> p (a b)"), Exp,
                            bias=0.0, scale=1.0 / 8.0)
                    if dt < 7:
                        kstep(2 * jj)
                        kstep(2 * jj + 1)
                    if jj in (6, 8, 10, 12):
                        den_drain((jj - 6) // 2)
                pvstep(JT - 2)
                pvstep(JT - 1)

                # stage PV PSUM to SBUF (frees psP) and take the reciprocal
                # off-path; the broadcast+mul happen during dt+1. The DVE
                # reciprocal has a ~3.3us fixed cost, so both heads'
                # denominators share one [1,1024] instruction.
                pvsb = []
                den2 = den2_bufs[dt % 2]
                for hb in range(2):
                    t = npool.tile([DH + 1, 512], F32, tag="pv",
                                   name=f"pvsb{dt}_{hb}")
                    nc.vector.tensor_copy(t[:], pv_ps[hb][:])
                    pvsb.append(t)
                    nc.vector.tensor_copy(den2[32 * hb:32 * hb + 1, :],
                                          t[DH:DH + 1, :])
                # reciprocal cost scales with free size only, so one [33,512]
                # instruction covers both heads (rows 1..31 are don't-care).
                rdivw = npool.tile([33, 512], BF, tag="n",
                                   name=f"rdivw{dt}")
                with nc.allow_low_precision(
                        reason="bf16 1/den: per-(h,q) scale, ~0.2%"):
                    nc.vector.reciprocal(rdivw[:], den2[:])
                rdiv = [rdivw[0:1, :], rdivw[32:33, :]]
                if dt < 7:
                    ps_d = psD.tile([128, 512], F32, tag="dd")
                else:
                    ps_d = None  # dt=7 drains in the tail from the psO pool
                den_prev = (dt, pvsb, rdiv, ps_d)

        # ---- output projection + residual + LayerNorm ----
        # O-proj chains front-load their kt=0..6 steps (probt for dt<7 is
        # long drained); the dt=7 den drain interleaves, and only the final
        # kt=7 step of each chain waits on probt(7).
        if not ln_plain:
            nc.gpsimd.dma_start(out=gamma_sb[:], in_=bcast_dram(hh["gamma"]))
            nc.gpsimd.dma_start(out=beta_sb[:], in_=bcast_dram(hh["beta"]))
        with tc.tile_pool(name="psO", bufs=3, space="PSUM") as psO:
            o_state = {}

            def o_front(it):
                xq_t = xqp.tile([128, 1024], F32, tag="xq",
                                name=f"xq{it}")
                nc.sync.dma_start(out=xq_t[:],
                                  in_=xq_ap[it * 128:(it + 1) * 128, :])
                ps_r = psO.tile([128, 2, 512], F32, tag="oo",
                                name=f"psr{it}")
                for mh in range(2):
                    for kt in range(7):
                        nc.tensor.matmul(
                            ps_r[:, mh, :],
                            probt[:, kt, it * 128:(it + 1) * 128],
                            wo_sb[:, kt, mh * 512:(mh + 1) * 512],
                            start=(kt == 0), stop=False)
                o_state[it] = (ps_r, xq_t)

            o_front(0)
            o_front(1)
            o_front(2)
            # drain dt=7's normalization using a psO bank
            d7, pvsb7, rdiv7, _ = den_prev
            ps_d7 = psO.tile([128, 512], F32, tag="ddt", bufs=1)
            den_prev = (d7, pvsb7, rdiv7, ps_d7)
            for step in range(4):
                den_drain(step)

            for it in range(IT):
                if it == 1:
                    o_front(3)
                ps_r, xq_t = o_state.pop(it)
                for mh in range(2):
                    nc.tensor.matmul(
                        ps_r[:, mh, :],
                        probt[:, 7, it * 128:(it + 1) * 128],
                        wo_sb[:, 7, mh * 512:(mh + 1) * 512],
                        start=False, stop=True)
                h_sb = lnp.tile([128, 1024], F32, tag="ln")
                nc.vector.tensor_add(h_sb[:],
                                     ps_r.rearrange("p a b -> p (a b)"),
                                     xq_t[:])
                stats = statp.tile([128, 2, 6], F32)
                nc.vector.bn_stats(stats[:, 0, :], h_sb[:, 0:512])
                nc.vector.bn_stats(stats[:, 1, :], h_sb[:, 512:1024])
                mv = statp.tile([128, 2], F32)
                nc.vector.bn_aggr(mv[:], stats[:])
                std = statp.tile([128, 1], F32)
                nc.scalar.activation(std[:], mv[:, 1:2], Sqrt,
                                     bias=eps_sb[:], scale=1.0)
                rstd = statp.tile([128, 1], F32)
                nc.vector.reciprocal(rstd[:], std[:])
                nmr = statp.tile([128, 1], F32)
                nc.vector.tensor_scalar(
                    nmr[:], rstd[:], mv[:, 0:1], -1.0,
                    op0=mybir.AluOpType.mult, op1=mybir.AluOpType.mult)
                # (h-mu)*rstd on the idle Scalar engine: h*rstd - mu*rstd
                t1 = lnp.tile([128, 1024], F32, tag="ln")
                nc.scalar.activation(
                    t1[:], h_sb[:], mybir.ActivationFunctionType.Identity,
                    bias=nmr[:], scale=rstd[:])
                if ln_plain:
                    out_t = t1
                else:
                    t2 = lnp.tile([128, 1024], F32, tag="ln")
                    nc.vector.tensor_mul(t2[:], t1[:], gamma_sb[:])
                    out_t = lnp.tile([128, 1024], F32, tag="ln")
                    nc.vector.tensor_add(out_t[:], t2[:], beta_sb[:])
                nc.sync.dma_start(y_ap[it * 128:(it + 1) * 128, :], out_t[:])


def build_module(split=True, masked=False, ln_plain=False):
    nc = bass.Bass("TRN2", target_bir_lowering=False, debug=False,
                   num_devices=N_CORES)
    hh = {
        "xt": nc.dram_tensor("xt", [D, L], BF, kind="ExternalInput"),
        "xq": nc.dram_tensor("xq", [Q, D], F32, kind="ExternalInput"),
        "wq": nc.dram_tensor("wq", [D, D], BF, kind="ExternalInput"),
        "wk": nc.dram_tensor("wk", [D, D], BF, kind="ExternalInput"),
        "wv": nc.dram_tensor("wv", [D, D], BF, kind="ExternalInput"),
        "wo": nc.dram_tensor("wo", [D, D], BF, kind="ExternalInput"),
        "bias": nc.dram_tensor("bias", [16, 128], F32, kind="ExternalInput"),
        "gamma": nc.dram_tensor("gamma", [D], BF, kind="ExternalInput"),
        "beta": nc.dram_tensor("beta", [D], BF, kind="ExternalInput"),
        "y": nc.dram_tensor("y", [Q, D], F32, kind="ExternalOutput"),
    }
    with tile.TileContext(nc) as tc:
        _emit(nc, tc, hh, masked, ln_plain)
    if split:
        _split_waits(nc, 1)
    return nc


_CACHE = {}
_LN_PLAIN = False


def get_module(masked=False):
    key = ("nc", masked, _LN_PLAIN)
    if key not in _CACHE:
        _CACHE[key] = build_module(masked=masked, ln_plain=_LN_PLAIN)
    return _CACHE[key]


def prep_inputs(x, mask, w_q, w_k, w_v, w_o, ln_gamma, ln_beta):
    x = np.asarray(x, dtype=np.float32)
    mask = np.asarray(mask)
    shared = {
        "wq": np.ascontiguousarray(
            np.asarray(w_q, np.float32).transpose(1, 0, 2).reshape(D, D)
        ).astype(BF16),
        "wk": np.ascontiguousarray(
            np.asarray(w_k, np.float32).transpose(1, 0, 2).reshape(D, D)
        ).astype(BF16),
        "wv": np.ascontiguousarray(
            np.asarray(w_v, np.float32).transpose(1, 0, 2).reshape(D, D)
        ).astype(BF16),
        "wo": np.asarray(w_o, np.float32).reshape(D, D).astype(BF16),
        "gamma": np.asarray(ln_gamma, np.float32).astype(BF16),
        "beta": np.asarray(ln_beta, np.float32).astype(BF16),
    }
    in_maps = []
    for c in range(N_CORES):
        b, q0 = c // 4, (c % 4) * Q
        perm = np.r_[q0:L, 0:q0]
        xb = x[b][perm]                       # rotated: q-chunk first
        m = {
            "xt": np.ascontiguousarray(xb.T).astype(BF16),
            "xq": np.ascontiguousarray(x[b, q0:q0 + Q, :]),
            "bias": np.where(mask[b][perm], 0.0, -1e9).astype(
                np.float32).reshape(16, 128),
        }
        m.update(shared)
        in_maps.append(m)
    masked = not bool(mask.all())
    global _LN_PLAIN
    _LN_PLAIN = bool(np.all(np.asarray(ln_gamma) == 1.0)
                     and np.all(np.asarray(ln_beta) == 0.0))
    return in_maps, masked


def assemble(results):
    out = np.empty((B, L, D), dtype=np.float32)
    for c in range(N_CORES):
        b, q0 = c // 4, (c % 4) * Q
        out[b, q0:q0 + Q, :] = results[c]["y"]
    return out


def run(in_maps, masked=False, **kwargs):
    nc = get_module(masked)
    return bass_utils.run_bass_kernel_spmd(
        nc, in_maps, core_ids=list(range(N_CORES)), **kwargs)


def kernel(x, mask, w_q, w_k, w_v, w_o, ln_gamma, ln_beta):
    in_maps, masked = prep_inputs(x, mask, w_q, w_k, w_v, w_o,
                                  ln_gamma, ln_beta)
    res = run(in_maps, masked)
    return assemble(res.results)
